# revision 4
# baseline (speedup 1.0000x reference)
"""Cross-attention (two-stream) Bass kernel for Trainium2, SPMD over 8 cores.

Problem (hardcoded shapes): two streams x_A, x_B of [B=2, C=128, H=64, W=64].
Per stream: GroupNorm(16 groups) -> qkv = conv1x1 (w_qkv [3C, C], split
q/k/v WITHIN each head: row n*96+{0..31}=q, +{32..63}=k, +{64..95}=v) ->
cross attention (queries from the OTHER stream), scale 1/sqrt(C),
softmax over keys -> conv1x1 out proj + bias + residual (on normalized x).

Sharding: 8 cores = (out_stream in {A,B}) x (batch in {0,1}) x (query-half
in {0,1}).  Each core receives the full spatial extent of both streams'
inputs for its batch (columns permuted so its query half is first),
computes GN + projections + 4-head attention for its 2048 query columns,
and writes a [128, 2048] slice of the output.  No collectives needed.

On-device layout: channels on partitions, spatial on free dim.
Attention is computed transposed (scores^T = [k, q]) so softmax's
denominator is obtained for free via a ones-column appended to the
V^T stationary operand, and no transposes of the big attention matrix
are ever needed.
"""

import math
import numpy as np

C = 128
NH = 4
D = 32          # head dim
G = 16          # groupnorm groups
B = 2
H = 64
W = 64
L = H * W       # 4096
HALF = L // 2   # 2048
QB = 512        # query block (one PSUM bank)
NQB = HALF // QB   # 4
KC = 128        # key chunk (contraction tile for AV)
NKC = L // KC      # 32
EPS = 1e-5
SCALE = 1.0 / math.sqrt(C)
N_CORES = 8

_PROGRAM = None


def _row_indices(kind: int) -> np.ndarray:
    # w_qkv row index for (head n, dim i) of q (kind 0) / k (1) / v (2)
    return np.array([n * (3 * D) + kind * D + i for n in range(NH) for i in range(D)])


def _build_program():
    import concourse.bacc as bacc
    import concourse.tile as tile
    from concourse import mybir
    from contextlib import ExitStack

    f32 = mybir.dt.float32
    bf16 = mybir.dt.bfloat16

    nc = bacc.Bacc()

    # ---- DRAM I/O ----
    d_xq = nc.declare_dram_parameter("xq", [C, L], f32, isOutput=False)
    d_xkv = nc.declare_dram_parameter("xkv", [C, L], f32, isOutput=False)
    d_wqT = nc.declare_dram_parameter("wqT", [C, C], f32, isOutput=False)
    d_wkT = nc.declare_dram_parameter("wkT", [C, C], f32, isOutput=False)
    d_wvT = nc.declare_dram_parameter("wvT", [C, NH * 33], f32, isOutput=False)
    d_woT = nc.declare_dram_parameter("woT", [C, C], f32, isOutput=False)
    d_bout = nc.declare_dram_parameter("bout", [C, 1], f32, isOutput=False)
    d_gnwq = nc.declare_dram_parameter("gnwq", [C, 1], f32, isOutput=False)
    d_gnbq = nc.declare_dram_parameter("gnbq", [C, 1], f32, isOutput=False)
    d_gnwk = nc.declare_dram_parameter("gnwk", [C, 1], f32, isOutput=False)
    d_gnbk = nc.declare_dram_parameter("gnbk", [C, 1], f32, isOutput=False)
    d_gnmask = nc.declare_dram_parameter("gnmask", [C, C], f32, isOutput=False)
    d_maskB = nc.declare_dram_parameter("maskB", [C, 2 * C], f32, isOutput=False)
    d_out = nc.declare_dram_parameter("out", [C, HALF], f32, isOutput=True)

    with tile.TileContext(nc) as tc, ExitStack() as ctx:
        P = ctx.enter_context(tc.tile_pool(name="persist", bufs=1))
        WRK = ctx.enter_context(tc.tile_pool(name="work", bufs=2))
        ET = ctx.enter_context(tc.tile_pool(name="expw", bufs=4))
        PS_S = ctx.enter_context(tc.tile_pool(name="ps_s", bufs=2, space="PSUM"))
        PS_AV = ctx.enter_context(tc.tile_pool(name="ps_av", bufs=2, space="PSUM"))
        PS_O = ctx.enter_context(tc.tile_pool(name="ps_o", bufs=2, space="PSUM"))

        # ---- load constants / weights ----
        xq = P.tile([C, L], f32, tag="xq")
        xkv = P.tile([C, L], f32, tag="xkv")
        # chunked loads so groupnorm stats can start on early chunks
        NDMA = 4
        for j in range(NDMA):
            s = L // NDMA
            nc.sync.dma_start(out=xq[:, j * s:(j + 1) * s], in_=d_xq[:, j * s:(j + 1) * s])
            nc.sync.dma_start(out=xkv[:, j * s:(j + 1) * s], in_=d_xkv[:, j * s:(j + 1) * s])

        def load(dram, shape, tag):
            t = P.tile(shape, f32, tag=tag)
            nc.sync.dma_start(out=t[:], in_=dram[:])
            return t

        wqT = load(d_wqT, [C, C], "wqT")
        wkT = load(d_wkT, [C, C], "wkT")
        wvT = load(d_wvT, [C, NH * 33], "wvT")
        woT = load(d_woT, [C, C], "woT")
        bout = load(d_bout, [C, 1], "bout")
        gnwq = load(d_gnwq, [C, 1], "gnwq")
        gnbq = load(d_gnbq, [C, 1], "gnbq")
        gnwk = load(d_gnwk, [C, 1], "gnwk")
        gnbk = load(d_gnbk, [C, 1], "gnbk")
        gnmask = load(d_gnmask, [C, C], "gnmask")
        maskB = load(d_maskB, [C, 2 * C], "maskB")

        eps_t = P.tile([C, 1], f32, tag="eps")
        nc.vector.memset(eps_t[:], EPS)

        # bf16 casts of matmul weights
        wqT_bf = P.tile([C, C], bf16, tag="wqT_bf")
        wkT_bf = P.tile([C, C], bf16, tag="wkT_bf")
        wvT_bf = P.tile([C, NH * 33], bf16, tag="wvT_bf")
        woT_bf = P.tile([C, C], bf16, tag="woT_bf")
        nc.vector.tensor_copy(out=wqT_bf[:], in_=wqT[:])
        nc.vector.tensor_copy(out=wkT_bf[:], in_=wkT[:])
        nc.vector.tensor_copy(out=wvT_bf[:], in_=wvT[:])
        nc.vector.tensor_copy(out=woT_bf[:], in_=woT[:])

        # ---- groupnorm: returns per-partition scale/shift [C,1] ----
        def gn_scale_shift(x_sb, gnw, gnb, tag):
            NS = L // 512  # 8 subgroups for bn_stats
            stats = P.tile([C, NS, 6], f32, tag=f"st_{tag}")
            for j in range(NS):
                nc.vector.bn_stats(out=stats[:, j, :], in_=x_sb[:, j * 512:(j + 1) * 512])
            mv = P.tile([C, 2], f32, tag=f"mv_{tag}")
            nc.vector.bn_aggr(out=mv[:], in_=stats[:])
            # S = [mean, var + mean^2] per partition
            S = P.tile([C, 2], f32, tag=f"S_{tag}")
            nc.vector.tensor_copy(out=S[:, 0:1], in_=mv[:, 0:1])
            nc.vector.tensor_mul(out=S[:, 1:2], in0=mv[:, 0:1], in1=mv[:, 0:1])
            nc.vector.tensor_add(out=S[:, 1:2], in0=S[:, 1:2], in1=mv[:, 1:2])
            # group-average across the 8 partitions of each group (PE)
            gps = PS_O.tile([C, QB], f32, tag="po")
            nc.tensor.matmul(gps[:, 0:2], lhsT=gnmask[:], rhs=S[:])
            gmean = P.tile([C, 1], f32, tag=f"gm_{tag}")
            nc.vector.tensor_copy(out=gmean[:], in_=gps[:, 0:1])
            m2 = P.tile([C, 1], f32, tag=f"m2_{tag}")
            nc.vector.tensor_mul(out=m2[:], in0=gmean[:], in1=gmean[:])
            var = P.tile([C, 1], f32, tag=f"var_{tag}")
            nc.vector.tensor_sub(out=var[:], in0=gps[:, 1:2], in1=m2[:])
            # rsig = 1/sqrt(var+eps)
            nc.scalar.activation(out=var[:], in_=var[:],
                                 func=mybir.ActivationFunctionType.Sqrt,
                                 bias=eps_t[:], scale=1.0)
            nc.vector.reciprocal(out=var[:], in_=var[:])
            sc = P.tile([C, 1], f32, tag=f"sc_{tag}")
            sh = P.tile([C, 1], f32, tag=f"sh_{tag}")
            nc.vector.tensor_mul(out=sc[:], in0=var[:], in1=gnw[:])
            nc.vector.tensor_mul(out=sh[:], in0=gmean[:], in1=sc[:])
            nc.vector.tensor_sub(out=sh[:], in0=gnb[:], in1=sh[:])
            return sc, sh

        sc_k, sh_k = gn_scale_shift(xkv, gnwk, gnbk, "k")
        sc_q, sh_q = gn_scale_shift(xq, gnwq, gnbq, "q")

        # apply GN
        xgn_kv = P.tile([C, L], f32, tag="xgn_kv")
        nc.vector.tensor_scalar(out=xgn_kv[:], in0=xkv[:], scalar1=sc_k[:],
                                scalar2=sh_k[:], op0=mybir.AluOpType.mult,
                                op1=mybir.AluOpType.add)
        xgn_kv_bf = P.tile([C, L], bf16, tag="xgn_kv_bf")
        nc.vector.tensor_copy(out=xgn_kv_bf[:], in_=xgn_kv[:])
        # query stream: only the first half is needed, straight to bf16
        xgn_q_bf = P.tile([C, HALF], bf16, tag="xgn_q_bf")
        nc.vector.tensor_scalar(out=xgn_q_bf[:], in0=xq[:, 0:HALF], scalar1=sc_q[:],
                                scalar2=sh_q[:], op0=mybir.AluOpType.mult,
                                op1=mybir.AluOpType.add)
        # residual (+ output bias, added once here)
        res_pre = P.tile([C, HALF], f32, tag="res_pre")
        nc.vector.tensor_scalar_add(out=res_pre[:], in0=xgn_kv[:, 0:HALF], scalar1=bout[:])

        # ---- projections ----
        kbf = P.tile([C, L], bf16, tag="kbf")
        for j in range(L // QB):
            ps = PS_O.tile([C, QB], f32, tag="po")
            nc.tensor.matmul(ps[:], lhsT=wkT_bf[:], rhs=xgn_kv_bf[:, j * QB:(j + 1) * QB])
            nc.vector.tensor_copy(out=kbf[:, j * QB:(j + 1) * QB], in_=ps[:])
        qbf = P.tile([C, HALF], bf16, tag="qbf")
        for j in range(HALF // QB):
            ps = PS_O.tile([C, QB], f32, tag="po")
            nc.tensor.matmul(ps[:], lhsT=wqT_bf[:], rhs=xgn_q_bf[:, j * QB:(j + 1) * QB])
            nc.vector.tensor_copy(out=qbf[:, j * QB:(j + 1) * QB], in_=ps[:])
        # v^T, chunked over spatial: vt[:, c, 33h:33h+32] = V^T for head h,
        # vt[:, c, 33h+32] = ones (softmax denominator trick)
        vt = P.tile([C, NKC, NH * 33], bf16, tag="vt")
        for c in range(NKC):
            ps = PS_O.tile([C, QB], f32, tag="po")
            nc.tensor.matmul(ps[:, 0:NH * 33], lhsT=xgn_kv_bf[:, c * KC:(c + 1) * KC],
                             rhs=wvT_bf[:])
            nc.vector.tensor_copy(out=vt[:, c, :], in_=ps[:, 0:NH * 33])
        for h in range(NH):
            nc.vector.memset(vt[:, :, 33 * h + 32:33 * h + 33], 1.0)

        # denominator staging tiles (only rows 32 and 96 are ever non-zero)
        dn = []
        for p in range(2):
            t = P.tile([C, QB], f32, tag=f"dn{p}")
            nc.vector.memset(t[:], 0.0)
            dn.append(t)

        out_sb = P.tile([C, HALF], f32, tag="out_sb")

        # ---- attention main loop ----
        for qb in range(NQB):
            qs = slice(qb * QB, (qb + 1) * QB)
            av = [PS_AV.tile([C, QB], f32, tag="av", name=f"av{qb}_{i}") for i in range(2)]
            for kc in range(NKC):
                for p in range(2):
                    st = PS_S.tile([C, 2 * QB], f32, tag="st")
                    for i in range(2):
                        h = 2 * p + i
                        tp = (96, 0) if h == 3 else None
                        nc.tensor.matmul(st[:, i * QB:(i + 1) * QB],
                                         lhsT=kbf[32 * h:32 * (h + 1), kc * KC:(kc + 1) * KC],
                                         rhs=qbf[32 * h:32 * (h + 1), qs],
                                         tile_position=tp)
                    et = ET.tile([C, 2 * QB], bf16, tag="et")
                    nc.scalar.activation(out=et[:], in_=st[:],
                                         func=mybir.ActivationFunctionType.Exp,
                                         scale=SCALE)
                    for i in range(2):
                        h = 2 * p + i
                        nc.tensor.matmul(av[p][64 * i:64 * i + 33, :],
                                         lhsT=vt[:, kc, 33 * h:33 * (h + 1)],
                                         rhs=et[:, i * QB:(i + 1) * QB],
                                         start=(kc == 0), stop=(kc == NKC - 1),
                                         skip_group_check=True)
            # epilogue for this query block
            attn_bf = WRK.tile([C, QB], bf16, tag="attn")
            for p in range(2):
                nc.vector.tensor_copy(out=dn[p][32:33, :], in_=av[p][32:33, :])
                nc.vector.tensor_copy(out=dn[p][96:97, :], in_=av[p][96:97, :])
                nc.vector.tensor_copy(out=attn_bf[64 * p:64 * p + 32, :], in_=av[p][0:32, :])
                nc.vector.tensor_copy(out=attn_bf[64 * p + 32:64 * p + 64, :], in_=av[p][64:96, :])
            ps_b = PS_O.tile([C, QB], f32, tag="po")
            nc.tensor.matmul(ps_b[:], lhsT=maskB[:, 0:C], rhs=dn[0][:], start=True, stop=False)
            nc.tensor.matmul(ps_b[:], lhsT=maskB[:, C:2 * C], rhs=dn[1][:], start=False, stop=True)
            rec = WRK.tile([C, QB], f32, tag="rec")
            nc.vector.reciprocal(out=rec[:], in_=ps_b[:])
            attn_n = WRK.tile([C, QB], bf16, tag="attn_n")
            nc.vector.tensor_mul(out=attn_n[:], in0=attn_bf[:], in1=rec[:])
            ps_o = PS_O.tile([C, QB], f32, tag="po")
            nc.tensor.matmul(ps_o[:], lhsT=woT_bf[:], rhs=attn_n[:])
            nc.vector.tensor_add(out=out_sb[:, qs], in0=ps_o[:], in1=res_pre[:, qs])
            nc.sync.dma_start(out=d_out[:, qs], in_=out_sb[:, qs])

    nc.compile()
    return nc


def get_program():
    global _PROGRAM
    if _PROGRAM is None:
        _PROGRAM = _build_program()
    return _PROGRAM


def make_in_maps(x_A, x_B, gn_w_A, gn_b_A, gn_w_B, gn_b_B,
                 w_qkv_A, w_qkv_B, w_out_A, b_out_A, w_out_B, b_out_B):
    """Build the 8 per-core input dicts. Core = s_out*4 + b*2 + half."""
    iq, ik, iv = _row_indices(0), _row_indices(1), _row_indices(2)

    gnmask = np.zeros((C, C), np.float32)
    for g in range(G):
        gnmask[g * 8:(g + 1) * 8, g * 8:(g + 1) * 8] = 1.0 / 8.0

    maskB = np.zeros((C, 2 * C), np.float32)
    # pair p's stationary: row 32 -> head 2p cols, row 96 -> head 2p+1 cols
    for p in range(2):
        maskB[32, p * C + 64 * p:p * C + 64 * p + 32] = 1.0
        maskB[96, p * C + 64 * p + 32:p * C + 64 * p + 64] = 1.0

    streams = {
        0: (x_A, gn_w_A, gn_b_A, w_qkv_A, w_out_A, b_out_A),   # out stream A
        1: (x_B, gn_w_B, gn_b_B, w_qkv_B, w_out_B, b_out_B),
    }
    in_maps = []
    for s_out in range(2):
        x_s, gnw_s, gnb_s, wqkv_s, wout_s, bout_s = streams[s_out]
        x_o, gnw_o, gnb_o, wqkv_o, _, _ = streams[1 - s_out]
        wq = np.ascontiguousarray(np.asarray(wqkv_o)[iq].T)          # [C, C]
        wk = np.ascontiguousarray(np.asarray(wqkv_s)[ik].T)
        wv = np.asarray(wqkv_s)[iv].T                                # [C, C]
        wvT = np.zeros((C, NH * 33), np.float32)
        for h in range(NH):
            wvT[:, 33 * h:33 * h + 32] = wv[:, 32 * h:32 * (h + 1)]
        wo = np.ascontiguousarray(np.asarray(wout_s).T)
        for b in range(B):
            xs = np.asarray(x_s)[b].reshape(C, L)
            xo = np.asarray(x_o)[b].reshape(C, L)
            for half in range(2):
                hs = slice(half * HALF, (half + 1) * HALF)
                ho = slice((1 - half) * HALF, (2 - half) * HALF)
                in_maps.append({
                    "xq": np.ascontiguousarray(np.concatenate([xo[:, hs], xo[:, ho]], axis=1)),
                    "xkv": np.ascontiguousarray(np.concatenate([xs[:, hs], xs[:, ho]], axis=1)),
                    "wqT": wq, "wkT": wk, "wvT": wvT, "woT": wo,
                    "bout": np.asarray(bout_s, np.float32).reshape(C, 1),
                    "gnwq": np.asarray(gnw_o, np.float32).reshape(C, 1),
                    "gnbq": np.asarray(gnb_o, np.float32).reshape(C, 1),
                    "gnwk": np.asarray(gnw_s, np.float32).reshape(C, 1),
                    "gnbk": np.asarray(gnb_s, np.float32).reshape(C, 1),
                    "gnmask": gnmask, "maskB": maskB,
                })
    return in_maps


def assemble(results):
    """results: list of 8 dicts with 'out' [C, HALF] -> (out_A, out_B)."""
    outs = []
    for s_out in range(2):
        o = np.zeros((B, C, L), np.float32)
        for b in range(B):
            for half in range(2):
                core = s_out * 4 + b * 2 + half
                o[b, :, half * HALF:(half + 1) * HALF] = results[core]["out"]
        outs.append(o.reshape(B, C, H, W))
    return tuple(outs)


def kernel(**inputs):
    from concourse.bass_utils import run_bass_kernel_spmd
    nc = get_program()
    in_maps = make_in_maps(**inputs)
    res = run_bass_kernel_spmd(nc, in_maps, list(range(N_CORES)))
    return assemble(res.results)
